# revision 22
# baseline (speedup 1.0000x reference)
"""HardTripletloss kernel for 8x Trainium2 NeuronCores (Bass, SPMD).

Strategy (2 row-groups x 4 feature-quarters, v4):
  - img is [49, 1048576] fp32; row 0 = anchor, rows 1:17 positives, 17:49 negatives.
  - 8 cores = 2 row-groups x 4 D-quarters.  Cores 0-3 take rows {0..24} (anchor
    + first 24), cores 4-7 take rows {0, 25..48}; each core gets one contiguous
    D/4 = 262144 feature quarter.  25 rows/core instead of 49 halves the
    per-row instruction count, amortizing fixed per-instruction overheads
    (DVE ~230ns, ScalarE ~340ns+280ns accum-read) over 2048-element rows.
  - Per-core layout: D_quarter = 262144 = 128 * 2048.  SBUF X[p, r*2048 + j]
    = shard[r, p*2048 + j]: the feature dim is split 128 ways onto partitions,
    so the anchor's chunk (X[p, 0:2048]) lives in the SAME partition as every
    other row's chunk — per-row dots need no cross-partition movement.
  - One SWDGE dma_start per row (fp32->bf16 cast in flight, 16 DMA engines,
    8KB contiguous reads per descriptor): compute follows the load row by row,
    so the post-load tail is a single row's compute (~2.6us).
  - DVE scalar_tensor_tensor(row * anchor, accum_out) -> dot partials [128,1]
    fp32 for rows 1..24, plus row 1's square as a self-dot (engine balance).
    ScalarE activation(Square, accum_out) -> sqnorm partials for rows 0 (the
    anchor — its squared norm) and 2..24.
  - Cores export dots [128,24] + sqs [128,25]; the host sums partials (fp64)
    across partitions and cores and runs the tiny cos/top-k/clamp/mean
    epilogue (anchor norm comes from row-group-0 cores only).
"""

from contextlib import ExitStack

import numpy as np

N_ROWS = 49
D = 1048576
N_CORES = 8
N_Q = 4                  # feature quarters
D_SHARD = D // N_Q       # 262144
P = 128                  # partitions; feature-split within a core
J = D_SHARD // P         # 2048 contiguous features per (row, partition)
R = 25                   # rows per core (anchor + 24)

MARGIN = 0.3
K_POS = 4
K_NEG = 8
EPS = 1e-8

_CACHE: dict = {}


def _build():
    import concourse.bass as bass
    from concourse import mybir

    bf16 = mybir.dt.bfloat16
    f32 = mybir.dt.float32

    nc = bass.Bass("TRN2", target_bir_lowering=False, debug=False)
    img = nc.dram_tensor("img", [R, D_SHARD], f32, kind="ExternalInput")
    # Bulk results (rows finished well before the load ends) export early and
    # overlap the end of the load; the last row's results go out separately so
    # the post-load serial tail is minimal.  The last row's load and compute
    # are split into half-J pieces (halves summed on the host) to halve the
    # compute tail after its final half arrives.
    dots = nc.dram_tensor("dots", [P, R - 2], f32, kind="ExternalOutput")   # rows 1..23
    sqs = nc.dram_tensor("sqs", [P, R - 1], f32, kind="ExternalOutput")     # rows 0..23
    # row-24 halves: dots in cols 0-1, squares in cols 2-3, one late export
    tails = nc.dram_tensor("tails", [P, 4], f32, kind="ExternalOutput")

    H = J // 2
    LAST = R - 1

    with ExitStack() as ctx:
        x_sb = ctx.enter_context(nc.sbuf_tensor("x_sb", [P, R * J], bf16))
        dve_scr = ctx.enter_context(nc.sbuf_tensor("dve_scr", [P, J], bf16))
        act_scr = ctx.enter_context(nc.sbuf_tensor("act_scr", [P, J], bf16))
        dots_sb = ctx.enter_context(nc.sbuf_tensor("dots_sb", [P, R - 2], f32))
        sqs_sb = ctx.enter_context(nc.sbuf_tensor("sqs_sb", [P, R - 1], f32))
        tails_sb = ctx.enter_context(nc.sbuf_tensor("tails_sb", [P, 4], f32))

        load_sem = ctx.enter_context(nc.semaphore("load_sem"))  # +16 per load
        dve_sem = ctx.enter_context(nc.semaphore("dve_sem"))    # +1 bulk, +1 tail
        act_sem = ctx.enter_context(nc.semaphore("act_sem"))    # +1 bulk
        out_sem = ctx.enter_context(nc.semaphore("out_sem"))    # +16 per export
        block = ctx.enter_context(nc.Block())

        # (p, r, j) -> img[r, p*J + j]; dst partition p, free offset r*J + j
        img_v = img.ap().rearrange("r (p j) -> p r j", p=P)

        @block.gpsimd
        def _(gpsimd):
            for r in range(LAST):
                gpsimd.dma_start(
                    out=x_sb[:, r * J : (r + 1) * J],
                    in_=img_v[:, r : r + 1, :],
                ).then_inc(load_sem, 16)
            for h in range(2):
                gpsimd.dma_start(
                    out=x_sb[:, LAST * J + h * H : LAST * J + (h + 1) * H],
                    in_=img_v[:, LAST : LAST + 1, h * H : (h + 1) * H],
                ).then_inc(load_sem, 16)

        def stt(eng, scr, in0_sl, in1_sl, acc):
            return eng.scalar_tensor_tensor(
                out=scr,
                in0=in0_sl,
                scalar=1.0,
                in1=in1_sl,
                op0=mybir.AluOpType.mult,
                op1=mybir.AluOpType.mult,
                accum_out=acc,
            )

        @block.vector
        def _(vector):
            # dots rows 1..23, plus row 1's square (self-dot) for balance
            for i, (r, kind) in enumerate(
                [(1, "dot"), (1, "sq")] + [(r, "dot") for r in range(2, LAST)]
            ):
                if kind == "dot" or i == 1:
                    vector.wait_ge(load_sem, 16 * (r + 1))
                other = 0 if kind == "dot" else r
                acc = (
                    dots_sb[:, r - 1 : r] if kind == "dot" else sqs_sb[:, r : r + 1]
                )
                op = stt(
                    nc.vector,
                    dve_scr[:, :],
                    x_sb[:, r * J : (r + 1) * J],
                    x_sb[:, other * J : (other + 1) * J],
                    acc,
                )
                if r == LAST - 1 and kind == "dot":
                    op.then_inc(dve_sem, 1)  # bulk dots complete
            # last row in half-J pieces
            for h in range(2):
                vector.wait_ge(load_sem, 16 * (LAST + 1 + h))
                op = stt(
                    nc.vector,
                    dve_scr[:, 0:H],
                    x_sb[:, LAST * J + h * H : LAST * J + (h + 1) * H],
                    x_sb[:, h * H : (h + 1) * H],
                    tails_sb[:, h : h + 1],
                )
                if h == 1:
                    op.then_inc(dve_sem, 1)  # tail dots complete

        @block.scalar
        def _(scalar):
            # squares rows 0 (anchor) and 2..23
            for r in [0] + list(range(2, LAST)):
                scalar.wait_ge(load_sem, 16 * (r + 1))
                op = nc.scalar.activation(
                    out=act_scr[:, :],
                    in_=x_sb[:, r * J : (r + 1) * J],
                    func=mybir.ActivationFunctionType.Square,
                    accum_out=sqs_sb[:, r : r + 1],
                )
                if r == LAST - 1:
                    op.then_inc(act_sem, 1)  # bulk squares complete
            # last row in half-J pieces, then export both engines' tails in a
            # single DMA issued directly from ScalarE (HWDGE)
            for h in range(2):
                scalar.wait_ge(load_sem, 16 * (LAST + 1 + h))
                nc.scalar.activation(
                    out=act_scr[:, 0:H],
                    in_=x_sb[:, LAST * J + h * H : LAST * J + (h + 1) * H],
                    func=mybir.ActivationFunctionType.Square,
                    accum_out=tails_sb[:, 2 + h : 3 + h],
                )
            scalar.wait_ge(dve_sem, 2)
            scalar.dma_start(out=tails.ap(), in_=tails_sb[:, :]).then_inc(out_sem, 16)

        @block.sync
        def _(sync):
            sync.wait_ge(dve_sem, 1)
            sync.wait_ge(act_sem, 1)
            sync.dma_start(out=dots.ap(), in_=dots_sb[:, :]).then_inc(out_sem, 16)
            sync.dma_start(out=sqs.ap(), in_=sqs_sb[:, :]).then_inc(out_sem, 16)
            sync.wait_ge(out_sem, 48)

    nc.finalize()
    return nc


def _get_nc():
    if "nc" not in _CACHE:
        _CACHE["nc"] = _build()
    return _CACHE["nc"]


_G1_ROWS = np.r_[0, 25:49]  # rows for cores 4-7: anchor + negatives tail


def _run_spmd(img: np.ndarray, **kwargs):
    """Shard the full img, run the SPMD kernel, return BassKernelResults."""
    from concourse.bass_utils import run_bass_kernel_spmd

    assert img.shape == (N_ROWS, D), img.shape
    nc = _get_nc()
    in_maps = []
    for c in range(N_CORES):
        q = c % N_Q
        rows = slice(0, R) if c < N_Q else _G1_ROWS
        shard = np.ascontiguousarray(
            img[rows, q * D_SHARD : (q + 1) * D_SHARD], dtype=np.float32
        )
        assert shard.shape == (R, D_SHARD)
        in_maps.append({"img": shard})
    return run_bass_kernel_spmd(nc, in_maps, list(range(N_CORES)), **kwargs)


def _finish(results) -> np.ndarray:
    """Sum per-core partials and run the tiny triplet-loss epilogue on host."""
    s = np.zeros(N_ROWS, np.float64)
    q = np.zeros(N_ROWS, np.float64)
    for c in range(N_CORES):
        res = results[c]
        # rows 1..23 from the bulk export, row 24 from the half-row tail
        tails = res["tails"].astype(np.float64)  # [P, 4]: dot halves, sq halves
        d = np.concatenate(
            [res["dots"].astype(np.float64).sum(axis=0), [tails[:, 0:2].sum()]]
        )  # [24] = local rows 1..24
        sq = np.concatenate(
            [res["sqs"].astype(np.float64).sum(axis=0), [tails[:, 2:4].sum()]]
        )  # [25] = local rows 0..24
        if c < N_Q:
            s[1:R] += d
            q[0] += sq[0]  # anchor sq-norm: row-group-0 quarters only
            q[1:R] += sq[1:]
        else:
            s[R:] += d
            q[R:] += sq[1:]

    na_ = max(np.sqrt(q[0]), EPS)
    nb_ = np.maximum(np.sqrt(q[1:]), EPS)
    cos = s[1:] / (na_ * nb_)
    dist = 1.0 - cos
    d_p = dist[0:16]
    d_n = dist[16:48]
    mean_p = np.sort(d_p)[-K_POS:].mean()
    top_n = np.sort(d_n)[:K_NEG]
    loss = np.mean(np.maximum(mean_p - top_n + MARGIN, 0.0))
    return np.float32(loss)


def kernel(img: np.ndarray) -> np.ndarray:
    img = np.asarray(img)
    results = _run_spmd(img).results
    return _finish(results)


# revision 23
# speedup vs baseline: 1.0356x; 1.0356x over previous
"""HardTripletloss kernel for 8x Trainium2 NeuronCores (Bass, SPMD).

Strategy (2 row-groups x 4 feature-quarters, v4):
  - img is [49, 1048576] fp32; row 0 = anchor, rows 1:17 positives, 17:49 negatives.
  - 8 cores = 2 row-groups x 4 D-quarters.  Cores 0-3 take rows {0..24} (anchor
    + first 24), cores 4-7 take rows {0, 25..48}; each core gets one contiguous
    D/4 = 262144 feature quarter.  25 rows/core instead of 49 halves the
    per-row instruction count, amortizing fixed per-instruction overheads
    (DVE ~230ns, ScalarE ~340ns+280ns accum-read) over 2048-element rows.
  - Per-core layout: D_quarter = 262144 = 128 * 2048.  SBUF X[p, r*2048 + j]
    = shard[r, p*2048 + j]: the feature dim is split 128 ways onto partitions,
    so the anchor's chunk (X[p, 0:2048]) lives in the SAME partition as every
    other row's chunk — per-row dots need no cross-partition movement.
  - One SWDGE dma_start per row (fp32->bf16 cast in flight, 16 DMA engines,
    8KB contiguous reads per descriptor): compute follows the load row by row,
    so the post-load tail is a single row's compute (~2.6us).
  - DVE scalar_tensor_tensor(row * anchor, accum_out) -> dot partials [128,1]
    fp32 for rows 1..24, plus row 1's square as a self-dot (engine balance).
    ScalarE activation(Square, accum_out) -> sqnorm partials for rows 0 (the
    anchor — its squared norm) and 2..24.
  - Cores export dots [128,24] + sqs [128,25]; the host sums partials (fp64)
    across partitions and cores and runs the tiny cos/top-k/clamp/mean
    epilogue (anchor norm comes from row-group-0 cores only).
"""

from contextlib import ExitStack

import numpy as np

N_ROWS = 49
D = 1048576
N_CORES = 8
N_Q = 4                  # feature quarters
D_SHARD = D // N_Q       # 262144
P = 128                  # partitions; feature-split within a core
J = D_SHARD // P         # 2048 contiguous features per (row, partition)
R = 25                   # rows per core (anchor + 24)

MARGIN = 0.3
K_POS = 4
K_NEG = 8
EPS = 1e-8

_CACHE: dict = {}


def _build():
    import concourse.bass as bass
    from concourse import mybir

    bf16 = mybir.dt.bfloat16
    f32 = mybir.dt.float32

    nc = bass.Bass("TRN2", target_bir_lowering=False, debug=False)
    img = nc.dram_tensor("img", [R, D_SHARD], f32, kind="ExternalInput")
    # Bulk results (rows finished well before the load ends) export early and
    # overlap the end of the load; the last row's results go out separately so
    # the post-load serial tail is minimal.  The last row's load and compute
    # are split into half-J pieces (halves summed on the host) to halve the
    # compute tail after its final half arrives.
    dots = nc.dram_tensor("dots", [P, R - 2], f32, kind="ExternalOutput")   # rows 1..23
    sqs = nc.dram_tensor("sqs", [P, R - 1], f32, kind="ExternalOutput")     # rows 0..23
    # row-24 halves: dots in cols 0-1, squares in cols 2-3, one late export
    tails = nc.dram_tensor("tails", [P, 4], f32, kind="ExternalOutput")

    H = J // 2
    LAST = R - 1

    with ExitStack() as ctx:
        x_sb = ctx.enter_context(nc.sbuf_tensor("x_sb", [P, R * J], bf16))
        dve_scr = ctx.enter_context(nc.sbuf_tensor("dve_scr", [P, J], bf16))
        act_scr = ctx.enter_context(nc.sbuf_tensor("act_scr", [P, J], bf16))
        dots_sb = ctx.enter_context(nc.sbuf_tensor("dots_sb", [P, R - 2], f32))
        sqs_sb = ctx.enter_context(nc.sbuf_tensor("sqs_sb", [P, R - 1], f32))
        tails_sb = ctx.enter_context(nc.sbuf_tensor("tails_sb", [P, 4], f32))

        load_sem = ctx.enter_context(nc.semaphore("load_sem"))  # +16 per load
        dve_sem = ctx.enter_context(nc.semaphore("dve_sem"))    # +1 bulk, +1 tail
        act_sem = ctx.enter_context(nc.semaphore("act_sem"))    # +1 bulk
        out_sem = ctx.enter_context(nc.semaphore("out_sem"))    # +16 per export
        block = ctx.enter_context(nc.Block())

        # (p, r, j) -> img[r, p*J + j]; dst partition p, free offset r*J + j
        img_v = img.ap().rearrange("r (p j) -> p r j", p=P)

        @block.gpsimd
        def _(gpsimd):
            for r in range(LAST):
                gpsimd.dma_start(
                    out=x_sb[:, r * J : (r + 1) * J],
                    in_=img_v[:, r : r + 1, :],
                ).then_inc(load_sem, 16)
            for h in range(2):
                gpsimd.dma_start(
                    out=x_sb[:, LAST * J + h * H : LAST * J + (h + 1) * H],
                    in_=img_v[:, LAST : LAST + 1, h * H : (h + 1) * H],
                ).then_inc(load_sem, 16)

        def stt(eng, scr, in0_sl, in1_sl, acc):
            return eng.scalar_tensor_tensor(
                out=scr,
                in0=in0_sl,
                scalar=1.0,
                in1=in1_sl,
                op0=mybir.AluOpType.mult,
                op1=mybir.AluOpType.mult,
                accum_out=acc,
            )

        @block.vector
        def _(vector):
            # dots rows 1..23, plus row 1's square (self-dot) for balance
            for i, (r, kind) in enumerate(
                [(1, "dot"), (1, "sq")] + [(r, "dot") for r in range(2, LAST)]
            ):
                if kind == "dot" or i == 1:
                    vector.wait_ge(load_sem, 16 * (r + 1))
                other = 0 if kind == "dot" else r
                acc = (
                    dots_sb[:, r - 1 : r] if kind == "dot" else sqs_sb[:, r : r + 1]
                )
                op = stt(
                    nc.vector,
                    dve_scr[:, :],
                    x_sb[:, r * J : (r + 1) * J],
                    x_sb[:, other * J : (other + 1) * J],
                    acc,
                )
                if r == LAST - 1 and kind == "dot":
                    op.then_inc(dve_sem, 1)  # bulk dots complete
            # last row in half-J pieces
            for h in range(2):
                vector.wait_ge(load_sem, 16 * (LAST + 1 + h))
                op = stt(
                    nc.vector,
                    dve_scr[:, 0:H],
                    x_sb[:, LAST * J + h * H : LAST * J + (h + 1) * H],
                    x_sb[:, h * H : (h + 1) * H],
                    tails_sb[:, h : h + 1],
                )
                if h == 1:
                    op.then_inc(dve_sem, 1)  # tail dots complete

        @block.scalar
        def _(scalar):
            # squares rows 0 (anchor) and 2..23
            for r in [0] + list(range(2, LAST)):
                scalar.wait_ge(load_sem, 16 * (r + 1))
                op = nc.scalar.activation(
                    out=act_scr[:, :],
                    in_=x_sb[:, r * J : (r + 1) * J],
                    func=mybir.ActivationFunctionType.Square,
                    accum_out=sqs_sb[:, r : r + 1],
                )
                if r == LAST - 1:
                    op.then_inc(act_sem, 1)  # bulk squares complete
            # last row in half-J pieces, then export both engines' tails in a
            # single DMA issued directly from ScalarE (HWDGE)
            for h in range(2):
                scalar.wait_ge(load_sem, 16 * (LAST + 1 + h))
                nc.scalar.activation(
                    out=act_scr[:, 0:H],
                    in_=x_sb[:, LAST * J + h * H : LAST * J + (h + 1) * H],
                    func=mybir.ActivationFunctionType.Square,
                    accum_out=tails_sb[:, 2 + h : 3 + h],
                )
            scalar.wait_ge(dve_sem, 2)
            scalar.dma_start(out=tails.ap(), in_=tails_sb[:, :]).then_inc(out_sem, 16)

        @block.sync
        def _(sync):
            sync.wait_ge(dve_sem, 1)
            sync.wait_ge(act_sem, 1)
            sync.dma_start(out=dots.ap(), in_=dots_sb[:, :]).then_inc(out_sem, 16)
            sync.dma_start(out=sqs.ap(), in_=sqs_sb[:, :]).then_inc(out_sem, 16)
            # No explicit wait on out_sem: the block-end teardown DRAINs each
            # engine's DMA queue, which already covers the in-flight exports
            # (the tiny transfers complete ~2us after issue, well inside the
            # ~7.5us teardown).  Dropping the wait removes its serialization
            # from the critical path.

    nc.finalize()
    return nc


def _get_nc():
    if "nc" not in _CACHE:
        _CACHE["nc"] = _build()
    return _CACHE["nc"]


_G1_ROWS = np.r_[0, 25:49]  # rows for cores 4-7: anchor + negatives tail


def _run_spmd(img: np.ndarray, **kwargs):
    """Shard the full img, run the SPMD kernel, return BassKernelResults."""
    from concourse.bass_utils import run_bass_kernel_spmd

    assert img.shape == (N_ROWS, D), img.shape
    nc = _get_nc()
    in_maps = []
    for c in range(N_CORES):
        q = c % N_Q
        rows = slice(0, R) if c < N_Q else _G1_ROWS
        shard = np.ascontiguousarray(
            img[rows, q * D_SHARD : (q + 1) * D_SHARD], dtype=np.float32
        )
        assert shard.shape == (R, D_SHARD)
        in_maps.append({"img": shard})
    return run_bass_kernel_spmd(nc, in_maps, list(range(N_CORES)), **kwargs)


def _finish(results) -> np.ndarray:
    """Sum per-core partials and run the tiny triplet-loss epilogue on host."""
    s = np.zeros(N_ROWS, np.float64)
    q = np.zeros(N_ROWS, np.float64)
    for c in range(N_CORES):
        res = results[c]
        # rows 1..23 from the bulk export, row 24 from the half-row tail
        tails = res["tails"].astype(np.float64)  # [P, 4]: dot halves, sq halves
        d = np.concatenate(
            [res["dots"].astype(np.float64).sum(axis=0), [tails[:, 0:2].sum()]]
        )  # [24] = local rows 1..24
        sq = np.concatenate(
            [res["sqs"].astype(np.float64).sum(axis=0), [tails[:, 2:4].sum()]]
        )  # [25] = local rows 0..24
        if c < N_Q:
            s[1:R] += d
            q[0] += sq[0]  # anchor sq-norm: row-group-0 quarters only
            q[1:R] += sq[1:]
        else:
            s[R:] += d
            q[R:] += sq[1:]

    na_ = max(np.sqrt(q[0]), EPS)
    nb_ = np.maximum(np.sqrt(q[1:]), EPS)
    cos = s[1:] / (na_ * nb_)
    dist = 1.0 - cos
    d_p = dist[0:16]
    d_n = dist[16:48]
    mean_p = np.sort(d_p)[-K_POS:].mean()
    top_n = np.sort(d_n)[:K_NEG]
    loss = np.mean(np.maximum(mean_p - top_n + MARGIN, 0.0))
    return np.float32(loss)


def kernel(img: np.ndarray) -> np.ndarray:
    img = np.asarray(img)
    results = _run_spmd(img).results
    return _finish(results)


# revision 30
# speedup vs baseline: 1.0894x; 1.0519x over previous
"""HardTripletloss kernel for 8x Trainium2 NeuronCores (Bass, SPMD).

Strategy (2 row-groups x 4 feature-quarters, v4):
  - img is [49, 1048576] fp32; row 0 = anchor, rows 1:17 positives, 17:49 negatives.
  - 8 cores = 2 row-groups x 4 D-quarters.  Cores 0-3 take rows {0..24} (anchor
    + first 24), cores 4-7 take rows {0, 25..48}; each core gets one contiguous
    D/4 = 262144 feature quarter.  25 rows/core instead of 49 halves the
    per-row instruction count, amortizing fixed per-instruction overheads
    (DVE ~230ns, ScalarE ~340ns+280ns accum-read) over 2048-element rows.
  - Per-core layout: D_quarter = 262144 = 128 * 2048.  SBUF X[p, r*2048 + j]
    = shard[r, p*2048 + j]: the feature dim is split 128 ways onto partitions,
    so the anchor's chunk (X[p, 0:2048]) lives in the SAME partition as every
    other row's chunk — per-row dots need no cross-partition movement.
  - One SWDGE dma_start per row (fp32->bf16 cast in flight, 16 DMA engines,
    8KB contiguous reads per descriptor): compute follows the load row by row,
    so the post-load tail is a single row's compute (~2.6us).
  - DVE scalar_tensor_tensor(row * anchor, accum_out) -> dot partials [128,1]
    fp32 for rows 1..24, plus row 1's square as a self-dot (engine balance).
    ScalarE activation(Square, accum_out) -> sqnorm partials for rows 0 (the
    anchor — its squared norm) and 2..24.
  - Cores export dots [128,24] + sqs [128,25]; the host sums partials (fp64)
    across partitions and cores and runs the tiny cos/top-k/clamp/mean
    epilogue (anchor norm comes from row-group-0 cores only).
"""

from contextlib import ExitStack

import numpy as np

N_ROWS = 49
D = 1048576
N_CORES = 8
N_Q = 4                  # feature quarters
D_SHARD = D // N_Q       # 262144
P = 128                  # partitions; feature-split within a core
J = D_SHARD // P         # 2048 contiguous features per (row, partition)
R = 25                   # rows per core (anchor + 24)

MARGIN = 0.3
K_POS = 4
K_NEG = 8
EPS = 1e-8

_CACHE: dict = {}


def _build():
    import concourse.bass as bass
    from concourse import mybir

    bf16 = mybir.dt.bfloat16
    f32 = mybir.dt.float32

    nc = bass.Bass("TRN2", target_bir_lowering=False, debug=False)
    img = nc.dram_tensor("img", [R, D_SHARD], f32, kind="ExternalInput")
    # Bulk results (rows finished well before the load ends) export early and
    # overlap the end of the load; the last row's results go out separately so
    # the post-load serial tail is minimal.  The last row's load and compute
    # are split into half-J pieces (halves summed on the host) to halve the
    # compute tail after its final half arrives.
    dots = nc.dram_tensor("dots", [P, R - 2], f32, kind="ExternalOutput")   # rows 1..23
    sqs = nc.dram_tensor("sqs", [P, R - 1], f32, kind="ExternalOutput")     # rows 0..23
    # row-24 pieces: dots in cols 0-2, squares in cols 3-5, one late export.
    # The final piece is a quarter row so the post-load compute tail is small.
    tails = nc.dram_tensor("tails", [P, 6], f32, kind="ExternalOutput")

    H = J // 2
    Q = J // 4
    PIECES = [(0, H), (H, Q), (H + Q, Q)]  # (offset, length) within row 24
    LAST = R - 1

    with ExitStack() as ctx:
        x_sb = ctx.enter_context(nc.sbuf_tensor("x_sb", [P, R * J], bf16))
        dve_scr = ctx.enter_context(nc.sbuf_tensor("dve_scr", [P, J], bf16))
        act_scr = ctx.enter_context(nc.sbuf_tensor("act_scr", [P, J], bf16))
        dots_sb = ctx.enter_context(nc.sbuf_tensor("dots_sb", [P, R - 2], f32))
        sqs_sb = ctx.enter_context(nc.sbuf_tensor("sqs_sb", [P, R - 1], f32))
        tails_sb = ctx.enter_context(nc.sbuf_tensor("tails_sb", [P, 6], f32))

        load_sem = ctx.enter_context(nc.semaphore("load_sem"))  # +16 per load
        dve_sem = ctx.enter_context(nc.semaphore("dve_sem"))    # +1 bulk, +1 tail
        act_sem = ctx.enter_context(nc.semaphore("act_sem"))    # +1 bulk
        out_sem = ctx.enter_context(nc.semaphore("out_sem"))    # +16 per export
        block = ctx.enter_context(nc.Block())

        # (p, r, j) -> img[r, p*J + j]; dst partition p, free offset r*J + j
        img_v = img.ap().rearrange("r (p j) -> p r j", p=P)

        @block.gpsimd
        def _(gpsimd):
            for r in range(LAST):
                gpsimd.dma_start(
                    out=x_sb[:, r * J : (r + 1) * J],
                    in_=img_v[:, r : r + 1, :],
                ).then_inc(load_sem, 16)
            for off, ln in PIECES:
                gpsimd.dma_start(
                    out=x_sb[:, LAST * J + off : LAST * J + off + ln],
                    in_=img_v[:, LAST : LAST + 1, off : off + ln],
                ).then_inc(load_sem, 16)

        def stt(eng, scr, in0_sl, in1_sl, acc):
            return eng.scalar_tensor_tensor(
                out=scr,
                in0=in0_sl,
                scalar=1.0,
                in1=in1_sl,
                op0=mybir.AluOpType.mult,
                op1=mybir.AluOpType.mult,
                accum_out=acc,
            )

        @block.vector
        def _(vector):
            # dots rows 1..23, plus row 1's square (self-dot) for balance
            for i, (r, kind) in enumerate(
                [(1, "dot"), (1, "sq")] + [(r, "dot") for r in range(2, LAST)]
            ):
                if kind == "dot" or i == 1:
                    vector.wait_ge(load_sem, 16 * (r + 1))
                other = 0 if kind == "dot" else r
                acc = (
                    dots_sb[:, r - 1 : r] if kind == "dot" else sqs_sb[:, r : r + 1]
                )
                op = stt(
                    nc.vector,
                    dve_scr[:, :],
                    x_sb[:, r * J : (r + 1) * J],
                    x_sb[:, other * J : (other + 1) * J],
                    acc,
                )
                if r == LAST - 1 and kind == "dot":
                    op.then_inc(dve_sem, 1)  # bulk dots complete
            # last row in graduated pieces
            for i, (off, ln) in enumerate(PIECES):
                vector.wait_ge(load_sem, 16 * (LAST + 1 + i))
                op = stt(
                    nc.vector,
                    dve_scr[:, 0:ln],
                    x_sb[:, LAST * J + off : LAST * J + off + ln],
                    x_sb[:, off : off + ln],
                    tails_sb[:, i : i + 1],
                )
                if i == len(PIECES) - 1:
                    op.then_inc(dve_sem, 1)  # tail dots complete

        @block.scalar
        def _(scalar):
            # squares rows 0 (anchor) and 2..23
            for r in [0] + list(range(2, LAST)):
                scalar.wait_ge(load_sem, 16 * (r + 1))
                op = nc.scalar.activation(
                    out=act_scr[:, :],
                    in_=x_sb[:, r * J : (r + 1) * J],
                    func=mybir.ActivationFunctionType.Square,
                    accum_out=sqs_sb[:, r : r + 1],
                )
                if r == LAST - 1:
                    op.then_inc(act_sem, 1)  # bulk squares complete
            # last row in graduated pieces; sync exports the tails so ScalarE's
            # block end comes right after its last accumulator read
            for i, (off, ln) in enumerate(PIECES):
                scalar.wait_ge(load_sem, 16 * (LAST + 1 + i))
                op = nc.scalar.activation(
                    out=act_scr[:, 0:ln],
                    in_=x_sb[:, LAST * J + off : LAST * J + off + ln],
                    func=mybir.ActivationFunctionType.Square,
                    accum_out=tails_sb[:, 3 + i : 4 + i],
                )
                if i == len(PIECES) - 1:
                    op.then_inc(act_sem, 1)  # tail squares complete

        @block.sync
        def _(sync):
            sync.wait_ge(dve_sem, 1)
            sync.wait_ge(act_sem, 1)
            sync.dma_start(out=dots.ap(), in_=dots_sb[:, :]).then_inc(out_sem, 16)
            sync.dma_start(out=sqs.ap(), in_=sqs_sb[:, :]).then_inc(out_sem, 16)
            sync.wait_ge(dve_sem, 2)
            sync.wait_ge(act_sem, 2)
            sync.dma_start(out=tails.ap(), in_=tails_sb[:, :]).then_inc(out_sem, 16)
            # No explicit wait on out_sem: the block-end teardown DRAINs each
            # engine's DMA queue, which already covers the in-flight exports
            # (the tiny transfers complete ~2us after issue, well inside the
            # ~7.5us teardown).  Dropping the wait removes its serialization
            # from the critical path.

    nc.finalize()
    return nc


def _get_nc():
    if "nc" not in _CACHE:
        _CACHE["nc"] = _build()
    return _CACHE["nc"]


_G1_ROWS = np.r_[0, 25:49]  # rows for cores 4-7: anchor + negatives tail


def _run_spmd(img: np.ndarray, **kwargs):
    """Shard the full img, run the SPMD kernel, return BassKernelResults."""
    from concourse.bass_utils import run_bass_kernel_spmd

    assert img.shape == (N_ROWS, D), img.shape
    nc = _get_nc()
    in_maps = []
    for c in range(N_CORES):
        q = c % N_Q
        rows = slice(0, R) if c < N_Q else _G1_ROWS
        shard = np.ascontiguousarray(
            img[rows, q * D_SHARD : (q + 1) * D_SHARD], dtype=np.float32
        )
        assert shard.shape == (R, D_SHARD)
        in_maps.append({"img": shard})
    return run_bass_kernel_spmd(nc, in_maps, list(range(N_CORES)), **kwargs)


def _finish(results) -> np.ndarray:
    """Sum per-core partials and run the tiny triplet-loss epilogue on host."""
    s = np.zeros(N_ROWS, np.float64)
    q = np.zeros(N_ROWS, np.float64)
    for c in range(N_CORES):
        res = results[c]
        # rows 1..23 from the bulk export, row 24 from the half-row tail
        tails = res["tails"].astype(np.float64)  # [P, 6]: dot pieces, sq pieces
        d = np.concatenate(
            [res["dots"].astype(np.float64).sum(axis=0), [tails[:, 0:3].sum()]]
        )  # [24] = local rows 1..24
        sq = np.concatenate(
            [res["sqs"].astype(np.float64).sum(axis=0), [tails[:, 3:6].sum()]]
        )  # [25] = local rows 0..24
        if c < N_Q:
            s[1:R] += d
            q[0] += sq[0]  # anchor sq-norm: row-group-0 quarters only
            q[1:R] += sq[1:]
        else:
            s[R:] += d
            q[R:] += sq[1:]

    na_ = max(np.sqrt(q[0]), EPS)
    nb_ = np.maximum(np.sqrt(q[1:]), EPS)
    cos = s[1:] / (na_ * nb_)
    dist = 1.0 - cos
    d_p = dist[0:16]
    d_n = dist[16:48]
    mean_p = np.sort(d_p)[-K_POS:].mean()
    top_n = np.sort(d_n)[:K_NEG]
    loss = np.mean(np.maximum(mean_p - top_n + MARGIN, 0.0))
    return np.float32(loss)


def kernel(img: np.ndarray) -> np.ndarray:
    img = np.asarray(img)
    results = _run_spmd(img).results
    return _finish(results)
